# revision 1
# baseline (speedup 1.0000x reference)
"""Multi-head attention (B=4, S=2048, E=1024, H=16, D=64) on 8 Trainium2 cores.

Sharding: batch x head-group. Core c handles batch c//2 and heads
(c%2)*8 .. (c%2)*8+7. Each core computes its QKV projection slice, the
attention for its 8 heads, and a partial output projection; the host sums
the two partials per batch and adds out_b.

Device dataflow (per core), all attention math in transposed layout:
  phase A: qkT [1024, S] = Wqk x^T (+bias), v [S, 512] = x Wv^T (+bias)
  phase B: per head: S^T[t,s] = kT.T qT -> exp -> expT; AV with ones-augmented
           v gives out^T [64, s] and the softmax denominator for free;
           normalize and place into aT [512, S].
  phase C: y_part [S, 1024] = aT.T @ woT, streamed to DRAM.

Matmuls run as float32r (fp32 data, single-pass PE mode).
"""

from contextlib import ExitStack

import numpy as np

import concourse.bacc as bacc
import concourse.bass as bass
import concourse.mybir as mybir
import concourse.tile as tile
from concourse import bass_utils

B, S, E, H, D = 4, 2048, 1024, 16, 64
NCORES = 8
HC = H // 2          # heads per core
DV = HC * D          # v width per core (= out-proj contraction per core)
EO = E               # out-proj output width
SB = 512             # s-block (matmul moving free dim)
TGRP = 4             # t-chunks per exp group (psum banks per scores tile)

F32 = mybir.dt.float32
MM_DT = mybir.dt.float32r


def build_nc(S=S, E=E, HC=HC, D=D, EO=EO, mm_dt=MM_DT):
    EC = E // 128            # e-chunks (contraction tiles for projections)
    MQK = 2 * HC * D // 128  # qk output row chunks (first half q, second k)
    DVC = HC * D // 128      # aT partition chunks
    TC = S // 128            # t-chunks
    NSB = S // SB            # s-blocks
    assert S % SB == 0 and E % 128 == 0 and (2 * HC * D) % 128 == 0
    assert TC % TGRP == 0 and EO % SB == 0 and D == 64

    nc = bacc.Bacc("TRN2", target_bir_lowering=False, debug=False,
                   enable_asserts=False, num_devices=NCORES)

    xT_d = nc.dram_tensor("xT", [E, S], mm_dt, kind="ExternalInput").ap()
    wqk_d = nc.dram_tensor("wqk", [MQK, E, 128], mm_dt, kind="ExternalInput").ap()
    bqk_d = nc.dram_tensor("bqk", [MQK, 128], F32, kind="ExternalInput").ap()
    wv_d = nc.dram_tensor("wv", [E, HC * D], mm_dt, kind="ExternalInput").ap()
    bv_d = nc.dram_tensor("bv", [HC * D], F32, kind="ExternalInput").ap()
    wo_d = nc.dram_tensor("wo", [HC * D, EO], mm_dt, kind="ExternalInput").ap()
    ones_d = nc.dram_tensor("ones", [1], mm_dt, kind="ExternalInput").ap()
    y_d = nc.dram_tensor("y", [S, EO], F32, kind="ExternalOutput").ap()

    def r(ap):
        return ap

    with tile.TileContext(nc) as tc, ExitStack() as ctx:
        # pools that live across phases
        pqk = ctx.enter_context(tc.tile_pool(name="pqk", bufs=1))
        pv = ctx.enter_context(tc.tile_pool(name="pv", bufs=1))
        qk_sb = pqk.tile([128, MQK, S], mm_dt)     # [dpart, chunk, s]
        v_sb = pv.tile([128, TC, HC, D + 1], mm_dt)  # [tpart, tchunk, head, d+ones]

        # ---------------- phase A: QKV projection ----------------
        with ExitStack() as ctxA:
            px = ctxA.enter_context(tc.tile_pool(name="px", bufs=1))
            pw = ctxA.enter_context(tc.tile_pool(name="pw", bufs=2))
            pwv = ctxA.enter_context(tc.tile_pool(name="pwv", bufs=1))
            pb = ctxA.enter_context(tc.tile_pool(name="pb", bufs=1))
            psA = ctxA.enter_context(tc.tile_pool(name="psA", bufs=4, space="PSUM"))

            xt = px.tile([128, EC, S], mm_dt)
            for c in range(EC):
                nc.sync.dma_start(out=xt[:, c, :], in_=xT_d[c * 128:(c + 1) * 128, :])

            bqk_sb = pb.tile([128, MQK], F32)
            nc.sync.dma_start(out=bqk_sb, in_=bqk_d.rearrange("c p -> p c"))
            bv_sb = pb.tile([128, HC * D], F32)
            bv_bcast = bass.AP(tensor=bv_d.tensor, offset=bv_d.offset,
                               ap=[[0, 128]] + list(bv_d.ap))
            nc.sync.dma_start(out=bv_sb, in_=bv_bcast)

            wv_sb = pwv.tile([128, EC, HC * D], mm_dt)
            nc.sync.dma_start(out=wv_sb, in_=wv_d.rearrange("(c p) n -> p c n", p=128))

            # q^T and k^T: psum [m 128, s 512] accumulated over e-chunks
            for j in range(MQK):
                w_t = pw.tile([128, EC, 128], mm_dt, tag="wqk")
                nc.sync.dma_start(out=w_t, in_=wqk_d[j].rearrange("(c p) m -> p c m", p=128))
                for sb in range(NSB):
                    ps = psA.tile([128, SB], F32, tag="psA")
                    for c in range(EC):
                        nc.tensor.matmul(
                            ps, lhsT=r(w_t[:, c, :]),
                            rhs=r(xt[:, c, sb * SB:(sb + 1) * SB]),
                            start=(c == 0), stop=(c == EC - 1))
                    nc.vector.tensor_scalar_add(
                        out=qk_sb[:, j, sb * SB:(sb + 1) * SB], in0=ps,
                        scalar1=bqk_sb[:, j:j + 1])

            # v: psum [t 128, dv 512] accumulated over e-chunks
            # ones column via broadcast DMA (memset can't emit float32r)
            ones_bcast = bass.AP(tensor=ones_d.tensor, offset=ones_d.offset,
                                 ap=[[0, 128], [0, TC * HC], [1, 1]])
            nc.sync.dma_start(
                out=v_sb[:, :, :, D:D + 1].rearrange("p a b c -> p (a b) c"),
                in_=ones_bcast)
            for t in range(TC):
                ps = psA.tile([128, HC * D], F32, tag="psA")
                for c in range(EC):
                    nc.tensor.matmul(
                        ps, lhsT=r(xt[:, c, t * 128:(t + 1) * 128]),
                        rhs=r(wv_sb[:, c, :]),
                        start=(c == 0), stop=(c == EC - 1))
                nc.vector.tensor_add(
                    out=v_sb[:, t, :, 0:D],
                    in0=ps.rearrange("p (h d) -> p h d", h=HC),
                    in1=bv_sb.rearrange("p (h d) -> p h d", h=HC))

        # ---------------- phases B+C scratch ----------------
        with ExitStack() as ctxBC:
            pa = ctxBC.enter_context(tc.tile_pool(name="pa", bufs=1))
            aT_sb = pa.tile([128, DVC, S], mm_dt)   # [epart, echunk, s]

            with ExitStack() as ctxB:
                pexp = ctxB.enter_context(tc.tile_pool(name="pexp", bufs=1))
                poh = ctxB.enter_context(tc.tile_pool(name="poh", bufs=2))
                pbc = ctxB.enter_context(tc.tile_pool(name="pbc", bufs=1))
                pdram = ctxB.enter_context(tc.tile_pool(name="pdram", bufs=2, space="DRAM"))
                psSC = ctxB.enter_context(tc.tile_pool(name="psSC", bufs=1, space="PSUM"))
                psAV = ctxB.enter_context(tc.tile_pool(name="psAV", bufs=2, space="PSUM"))

                for h in range(HC):
                    g, odd = h // 2, h % 2
                    p0 = odd * 64
                    oh = poh.tile([128, S], F32, tag="oh")  # rows 0-63 out^T, row 64 denom
                    for sb in range(NSB):
                        expT = pexp.tile([128, TC, SB], mm_dt, tag="expT")
                        for grp in range(TC // TGRP):
                            sc = psSC.tile([128, TGRP, SB], F32, tag="sc")
                            for c4 in range(TGRP):
                                t = grp * TGRP + c4
                                nc.tensor.matmul(
                                    sc[:, c4, :],
                                    lhsT=r(qk_sb[p0:p0 + 64, MQK // 2 + g, t * 128:(t + 1) * 128]),
                                    rhs=r(qk_sb[p0:p0 + 64, g, sb * SB:(sb + 1) * SB]),
                                    start=True, stop=True)
                            nc.scalar.activation(
                                out=expT[:, grp * TGRP:(grp + 1) * TGRP, :], in_=sc,
                                func=mybir.ActivationFunctionType.Exp)
                        av = psAV.tile([128, SB], F32, tag="av")
                        for t in range(TC):
                            nc.tensor.matmul(
                                av[0:D + 1, :], lhsT=r(v_sb[:, t, h, :]),
                                rhs=r(expT[:, t, :]),
                                start=(t == 0), stop=(t == TC - 1))
                        nc.vector.tensor_copy(
                            out=oh[0:D + 1, sb * SB:(sb + 1) * SB], in_=av[0:D + 1, :])
                    # normalize: rows 0-63 /= row 64 (denominator).
                    # partition broadcast via DRAM round-trip (gpsimd
                    # partition_broadcast mis-broadcasts from partition 64 on HW)
                    nc.vector.reciprocal(out=oh[D:D + 1, :], in_=oh[D:D + 1, :])
                    dden = pdram.tile([1, S], F32, tag="dden")
                    nc.sync.dma_start(out=dden, in_=oh[D:D + 1, :])
                    rb = pbc.tile([128, S], F32, tag="rb")
                    dbc = bass.AP(tensor=dden.tensor, offset=dden.offset,
                                  ap=[[0, D]] + list(dden.ap[1:]))
                    nc.sync.dma_start(out=rb[0:D, :], in_=dbc)
                    if not odd:
                        nc.vector.tensor_mul(
                            out=aT_sb[0:64, g, :], in0=oh[0:D, :], in1=rb[0:D, :])
                    else:
                        ohn = pbc.tile([128, S], mm_dt, tag="ohn")
                        nc.vector.tensor_mul(out=ohn[0:D, :], in0=oh[0:D, :], in1=rb[0:D, :])
                        nc.sync.dma_start(out=aT_sb[64:128, g, :], in_=ohn[0:D, :])

            # ---------------- phase C: out projection ----------------
            with ExitStack() as ctxC:
                pwo = ctxC.enter_context(tc.tile_pool(name="pwo", bufs=1))
                py = ctxC.enter_context(tc.tile_pool(name="py", bufs=3))
                psC = ctxC.enter_context(tc.tile_pool(name="psC", bufs=4, space="PSUM"))

                wo_sb = pwo.tile([128, DVC, EO], mm_dt)
                nc.sync.dma_start(out=wo_sb, in_=wo_d.rearrange("(c p) o -> p c o", p=128))

                for st in range(S // 128):
                    y_t = py.tile([128, EO], F32, tag="y")
                    for ob in range(EO // SB):
                        ps = psC.tile([128, SB], F32, tag="psC")
                        for j in range(DVC):
                            nc.tensor.matmul(
                                ps, lhsT=r(aT_sb[:, j, st * 128:(st + 1) * 128]),
                                rhs=r(wo_sb[:, j, ob * SB:(ob + 1) * SB]),
                                start=(j == 0), stop=(j == DVC - 1))
                        nc.vector.tensor_copy(out=y_t[:, ob * SB:(ob + 1) * SB], in_=ps)
                    nc.sync.dma_start(out=y_d[st * 128:(st + 1) * 128, :], in_=y_t)

    nc.compile()
    return nc


_cache: dict = {}


def _get_nc():
    if "nc" not in _cache:
        _cache["nc"] = build_nc()
    return _cache["nc"]


def _shard_inputs(x_q, qkv_w, qkv_b, out_w):
    """Per-core input maps. Core c: batch c//2, head group c%2."""
    alpha = np.float32(D ** -0.5)
    in_maps = []
    MQK = 2 * HC * D // 128
    for c in range(NCORES):
        b, g2 = c // 2, c % 2
        hlo = g2 * HC * D
        wq = qkv_w[hlo:hlo + HC * D] * alpha
        wk = qkv_w[E + hlo:E + hlo + HC * D]
        wqk_rows = np.concatenate([wq, wk], axis=0)          # [2*HC*D, E]
        wqk = np.ascontiguousarray(
            wqk_rows.reshape(MQK, 128, E).transpose(0, 2, 1))  # [MQK, E, 128]
        bq = qkv_b[hlo:hlo + HC * D] * alpha
        bk = qkv_b[E + hlo:E + hlo + HC * D]
        bqk = np.concatenate([bq, bk]).reshape(MQK, 128)
        wv = np.ascontiguousarray(qkv_w[2 * E + hlo:2 * E + hlo + HC * D].T)  # [E, DV]
        bv = np.ascontiguousarray(qkv_b[2 * E + hlo:2 * E + hlo + HC * D])
        wo = np.ascontiguousarray(out_w[:, hlo:hlo + HC * D].T)  # [DV, EO]
        xT = np.ascontiguousarray(x_q[b].T)                      # [E, S]
        in_maps.append({
            "xT": xT, "wqk": wqk, "bqk": np.ascontiguousarray(bqk),
            "wv": wv, "bv": bv, "wo": wo,
            "ones": np.ones((1,), np.float32),
        })
    return in_maps


def kernel(x_q, qkv_w, qkv_b, out_w, out_b):
    import os
    os.environ["BASS_NEVER_TRACE"] = "1"  # axon NTFF hook module is absent here
    x_q = np.asarray(x_q, dtype=np.float32)
    qkv_w = np.asarray(qkv_w, dtype=np.float32)
    qkv_b = np.asarray(qkv_b, dtype=np.float32)
    out_w = np.asarray(out_w, dtype=np.float32)
    out_b = np.asarray(out_b, dtype=np.float32)

    nc = _get_nc()
    in_maps = _shard_inputs(x_q, qkv_w, qkv_b, out_w)
    res = bass_utils.run_bass_kernel_spmd(nc, in_maps, core_ids=list(range(NCORES)))
    parts = [r["y"] for r in res.results]
    y = np.empty((B, S, E), dtype=np.float32)
    for b in range(B):
        y[b] = parts[2 * b] + parts[2 * b + 1] + out_b
    return y



# revision 13
# speedup vs baseline: 1.1052x; 1.1052x over previous
"""Multi-head attention (B=4, S=2048, E=1024, H=16, D=64) on 8 Trainium2 cores.

Sharding: batch x head-group. Core c handles batch c//2 and heads
(c%2)*8 .. (c%2)*8+7. Each core computes its QKV projection slice, the
attention for its 8 heads, and a partial output projection; the host sums
the two partials per batch and adds out_b.

v2 layout: bf16 operand storage (PSUM accumulation stays fp32), phase B
re-pipelined so the Exp activations stream back-to-back on ACT while PE
fills with scores/AV/leftover-QK matmuls, and softmax normalization done
on-chip (reciprocal + PE broadcast matmul) instead of a DRAM round-trip.

Device dataflow (per core), attention in transposed layout:
  phase A: v [t,h,d] = x Wv^T + bv (ones cols padded for the denominator
           trick), then q^T/k^T chunks j=0,4 (head pair 0).
  phase B: per head, per 512-col s-block: S^T tiles (2 t-chunks per PSUM
           buffer) -> exp -> expT bf16; AV with ones-augmented v gives
           out^T and the denominator; reciprocal + broadcast-matmul +
           multiply writes normalized aT. Remaining QK chunks j=1,5,2,6,3,7
           are woven into the stream (head h emits chunk for pair h//2+1..).
  phase C: y[s,:] = aT^T @ Wo per 128-row tile, streamed to DRAM.
"""

from contextlib import ExitStack

import numpy as np

import concourse.bacc as bacc
import concourse.bass as bass
import concourse.mybir as mybir
import concourse.tile as tile
from concourse import bass_utils

B, S, E, H, D = 4, 2048, 1024, 16, 64
NCORES = 8
HC = H // 2          # heads per core
DV = HC * D          # v width per core (= out-proj contraction per core)
EO = E               # out-proj output width
SB = 512             # s-block width in phase B

F32 = mybir.dt.float32
F32R = mybir.dt.float32r
BF16 = mybir.dt.bfloat16

MQK = 2 * HC * D // 128  # 8 qk row chunks (first half q, second half k)
EC = E // 128            # 8 contraction chunks for projections
DVC = DV // 128          # 4 aT partition chunks (head pairs)
TC = S // 128            # 16 t-chunks
NSB = S // SB            # 4 s-blocks


def build_nc():
    nc = bacc.Bacc("TRN2", target_bir_lowering=False, debug=False,
                   enable_asserts=False, num_devices=NCORES)

    xT_d = nc.dram_tensor("xT", [E, S], BF16, kind="ExternalInput").ap()
    wqk_d = nc.dram_tensor("wqk", [MQK, E, 128], BF16, kind="ExternalInput").ap()
    bqk_d = nc.dram_tensor("bqk", [MQK, 128], F32, kind="ExternalInput").ap()
    wv_d = nc.dram_tensor("wv", [E, DV], BF16, kind="ExternalInput").ap()
    bv_d = nc.dram_tensor("bv", [DV], F32, kind="ExternalInput").ap()
    wo_d = nc.dram_tensor("wo", [DVC, 128, EO], BF16, kind="ExternalInput").ap()
    ones_r_d = nc.dram_tensor("ones_r", [1], F32R, kind="ExternalInput").ap()
    ones_b_d = nc.dram_tensor("ones_b", [1], BF16, kind="ExternalInput").ap()
    y_d = nc.dram_tensor("y", [S, EO], F32, kind="ExternalOutput").ap()

    with tile.TileContext(nc) as tc, ExitStack() as ctx:
        # ---- persistent SBUF ----
        pqk = ctx.enter_context(tc.tile_pool(name="pqk", bufs=1))
        pv = ctx.enter_context(tc.tile_pool(name="pv", bufs=1))
        pa = ctx.enter_context(tc.tile_pool(name="pa", bufs=1))
        pwo = ctx.enter_context(tc.tile_pool(name="pwo", bufs=1))
        pmisc = ctx.enter_context(tc.tile_pool(name="pmisc", bufs=1))
        px = ctx.enter_context(tc.tile_pool(name="px", bufs=1))
        pw = ctx.enter_context(tc.tile_pool(name="pw", bufs=2))
        pwv = ctx.enter_context(tc.tile_pool(name="pwv", bufs=1))

        qk_sb = pqk.tile([128, MQK, S], BF16)        # [dpart, chunk, s]
        v_sb = pv.tile([128, TC, HC, D + 1], BF16)   # [tpart, tc, h, d+ones]
        aT_sb = pa.tile([128, DVC, S], BF16)         # [pair rows, pair, s]
        wo_sb = pwo.tile([128, DVC, EO], BF16)
        ones_sel = pmisc.tile([128, D], F32R)        # bcast matmul weights
        bqk_sb = pmisc.tile([128, MQK], F32)
        bv_sb = pmisc.tile([128, DV], F32)
        xt = px.tile([128, EC, S], BF16)
        wv_sb = pwv.tile([128, EC, DV], BF16)

        # A-tail qk psum (alive through phase B)
        psQT = ctx.enter_context(tc.tile_pool(name="psQT", bufs=1, space="PSUM"))

        # ---- input DMAs ----
        for c in range(EC):
            nc.sync.dma_start(out=xt[:, c, :], in_=xT_d[c * 128:(c + 1) * 128, :])
        nc.sync.dma_start(out=wv_sb, in_=wv_d.rearrange("(c p) n -> p c n", p=128))
        nc.sync.dma_start(out=wo_sb, in_=wo_d.rearrange("c p o -> p c o"))
        nc.sync.dma_start(out=bqk_sb, in_=bqk_d.rearrange("c p -> p c"))
        bv_bcast = bass.AP(tensor=bv_d.tensor, offset=bv_d.offset,
                           ap=[[0, 128]] + list(bv_d.ap))
        nc.sync.dma_start(out=bv_sb, in_=bv_bcast)
        ones_sel_bcast = bass.AP(tensor=ones_r_d.tensor, offset=ones_r_d.offset,
                                 ap=[[0, 128], [0, D], [1, 1]])
        nc.sync.dma_start(out=ones_sel.rearrange("p (d o) -> p d o", o=1),
                          in_=ones_sel_bcast)
        # ones column of v (col D) for the softmax denominator
        ones_v = bass.AP(tensor=ones_b_d.tensor, offset=ones_b_d.offset,
                         ap=[[0, 128], [0, TC * HC], [1, 1]])
        nc.sync.dma_start(
            out=v_sb[:, :, :, D:D + 1].rearrange("p a b c -> p (a b) c"),
            in_=ones_v)

        def dma_wqk(j):
            w_t = pw.tile([128, EC, 128], BF16, tag="wqk")
            nc.sync.dma_start(out=w_t, in_=wqk_d[j].rearrange("(c p) m -> p c m", p=128))
            return w_t

        wt0 = dma_wqk(0)
        wt4 = dma_wqk(4)

        # ================= phase A head: v + qk chunks 0,4 =================
        with ExitStack() as ctxA:
            psV = ctxA.enter_context(tc.tile_pool(name="psV", bufs=2, space="PSUM"))
            psA = ctxA.enter_context(tc.tile_pool(name="psA", bufs=2, space="PSUM"))

            for t in range(TC):
                ps = psV.tile([128, DV], F32, tag="psV")
                for c in range(EC):
                    nc.tensor.matmul(
                        ps, lhsT=xt[:, c, t * 128:(t + 1) * 128],
                        rhs=wv_sb[:, c, :], start=(c == 0), stop=(c == EC - 1))
                nc.vector.tensor_add(
                    out=v_sb[:, t, :, 0:D],
                    in0=ps.rearrange("p (h d) -> p h d", h=HC),
                    in1=bv_sb.rearrange("p (h d) -> p h d", h=HC))

            for j, w_t in ((0, wt0), (4, wt4)):
                for sbb in range(NSB):
                    ps = psA.tile([128, SB], F32, tag="psA")
                    for c in range(EC):
                        nc.tensor.matmul(
                            ps, lhsT=w_t[:, c, :],
                            rhs=xt[:, c, sbb * SB:(sbb + 1) * SB],
                            start=(c == 0), stop=(c == EC - 1))
                    nc.vector.tensor_scalar_add(
                        out=qk_sb[:, j, sbb * SB:(sbb + 1) * SB], in0=ps,
                        scalar1=bqk_sb[:, j:j + 1])

        # ================= phase B: attention =================
        TAIL_JS = [1, 5, 2, 6, 3, 7]
        with ExitStack() as ctxB:
            pexp = ctxB.enter_context(tc.tile_pool(name="pexp", bufs=2))
            pohb = ctxB.enter_context(tc.tile_pool(name="pohb", bufs=2))
            psSC = ctxB.enter_context(tc.tile_pool(name="psSC", bufs=2, space="PSUM"))
            psBC = ctxB.enter_context(tc.tile_pool(name="psBC", bufs=1, space="PSUM"))
            psAV = ctxB.enter_context(tc.tile_pool(name="psAV", bufs=1, space="PSUM"))

            tail_state = {}

            def emit_tail_dma(h):
                if h < len(TAIL_JS):
                    j = TAIL_JS[h]
                    tail_state[j] = dma_wqk(j)

            def emit_tail_qk(h, half):
                """One 512-wide qk unit (j = TAIL_JS[h], half in 0..3)."""
                if h >= len(TAIL_JS):
                    return
                j = TAIL_JS[h]
                w_t = tail_state[j]
                ps = psQT.tile([128, SB], F32, tag="qt")
                for c in range(EC):
                    nc.tensor.matmul(
                        ps, lhsT=w_t[:, c, :],
                        rhs=xt[:, c, half * SB:(half + 1) * SB],
                        start=(c == 0), stop=(c == EC - 1))
                nc.vector.tensor_scalar_add(
                    out=qk_sb[:, j, half * SB:(half + 1) * SB], in0=ps,
                    scalar1=bqk_sb[:, j:j + 1])

            for h in range(HC):
                g, odd = h // 2, h % 2
                p0 = odd * 64            # q/k partition offset within chunk
                emit_tail_dma(h)
                for sb in range(NSB):
                    expT = pexp.tile([128, TC, SB], BF16, tag="expT")
                    for grp in range(TC // 2):
                        sc = psSC.tile([128, 2, SB], F32, tag="sc")
                        for i2 in range(2):
                            t = grp * 2 + i2
                            nc.tensor.matmul(
                                sc[:, i2, :],
                                lhsT=qk_sb[p0:p0 + 64, MQK // 2 + g,
                                           t * 128:(t + 1) * 128],
                                rhs=qk_sb[p0:p0 + 64, g, sb * SB:(sb + 1) * SB],
                                start=True, stop=True)
                        nc.scalar.activation(
                            out=expT[:, grp * 2:grp * 2 + 2, :], in_=sc,
                            func=mybir.ActivationFunctionType.Exp)
                        # weave one leftover-QK unit into the middle of
                        # each s-block so its PSUM drain overlaps
                        if grp == 3:
                            emit_tail_qk(h, sb)
                    av = psAV.tile([128, SB], F32, tag="av")
                    for t in range(TC):
                        nc.tensor.matmul(
                            av[0:D + 1, :],
                            lhsT=v_sb[:, t, h, :],
                            rhs=expT[:, t, :],
                            start=(t == 0), stop=(t == TC - 1))
                    ohb = pohb.tile([128, SB], F32, tag="ohb")
                    nc.vector.tensor_copy(out=ohb[0:D + 1, :], in_=av[0:D + 1, :])
                    rden = pohb.tile([128, SB], F32R, tag="rden")
                    with nc.allow_low_precision(reason="softmax denom recip"):
                        nc.vector.reciprocal(out=rden[D:D + 1, :],
                                             in_=ohb[D:D + 1, :])
                    bc = psBC.tile([128, SB], F32, tag="bc")
                    nc.tensor.matmul(
                        bc[0:D, :],
                        lhsT=ones_sel[D:D + 1, :],
                        rhs=rden[D:D + 1, :],
                        start=True, stop=True)
                    if not odd:
                        nc.vector.tensor_mul(
                            out=aT_sb[0:D, g, sb * SB:(sb + 1) * SB],
                            in0=ohb[0:D, :], in1=bc[0:D, :])
                    else:
                        ohn = pohb.tile([128, SB], BF16, tag="ohn")
                        nc.vector.tensor_mul(
                            out=ohn[0:D, :], in0=ohb[0:D, :], in1=bc[0:D, :])
                        nc.sync.dma_start(
                            out=aT_sb[64:128, g, sb * SB:(sb + 1) * SB],
                            in_=ohn[0:D, :])

        # ================= phase C: out projection =================
        with ExitStack() as ctxC:
            py = ctxC.enter_context(tc.tile_pool(name="py", bufs=3))
            psC = ctxC.enter_context(tc.tile_pool(name="psC", bufs=2, space="PSUM"))

            for st in range(S // 128):
                y_t = py.tile([128, EO], F32, tag="y")
                for ob in range(EO // SB):
                    ps = psC.tile([128, SB], F32, tag="psC")
                    for j in range(DVC):
                        nc.tensor.matmul(
                            ps, lhsT=aT_sb[:, j, st * 128:(st + 1) * 128],
                            rhs=wo_sb[:, j, ob * SB:(ob + 1) * SB],
                            start=(j == 0), stop=(j == DVC - 1))
                    if ob % 2 == 0:
                        nc.vector.tensor_copy(
                            out=y_t[:, ob * SB:(ob + 1) * SB], in_=ps)
                    else:
                        nc.scalar.activation(
                            out=y_t[:, ob * SB:(ob + 1) * SB], in_=ps,
                            func=mybir.ActivationFunctionType.Copy)
                nc.sync.dma_start(out=y_d[st * 128:(st + 1) * 128, :], in_=y_t)

    nc.compile()
    return nc


_cache: dict = {}


def _get_nc():
    if "nc" not in _cache:
        _cache["nc"] = build_nc()
    return _cache["nc"]


def _shard_inputs(x_q, qkv_w, qkv_b, out_w):
    """Per-core input maps. Core c: batch c//2, head group c%2."""
    bf16 = mybir.dt.np(BF16)
    alpha = np.float32(D ** -0.5)
    in_maps = []
    for c in range(NCORES):
        b, g2 = c // 2, c % 2
        hlo = g2 * DV
        wq = qkv_w[hlo:hlo + DV] * alpha
        wk = qkv_w[E + hlo:E + hlo + DV]
        wqk_rows = np.concatenate([wq, wk], axis=0)          # [2*DV, E]
        wqk = np.ascontiguousarray(
            wqk_rows.reshape(MQK, 128, E).transpose(0, 2, 1)).astype(bf16)
        bq = qkv_b[hlo:hlo + DV] * alpha
        bk = qkv_b[E + hlo:E + hlo + DV]
        bqk = np.ascontiguousarray(
            np.concatenate([bq, bk]).reshape(MQK, 128))
        wv = np.ascontiguousarray(
            qkv_w[2 * E + hlo:2 * E + hlo + DV].T).astype(bf16)   # [E, DV]
        bv = np.ascontiguousarray(qkv_b[2 * E + hlo:2 * E + hlo + DV])
        wo = np.ascontiguousarray(
            out_w[:, hlo:hlo + DV].T.reshape(DVC, 128, EO)).astype(bf16)
        xT = np.ascontiguousarray(x_q[b].T).astype(bf16)          # [E, S]
        in_maps.append({
            "xT": xT, "wqk": wqk, "bqk": bqk, "wv": wv, "bv": bv, "wo": wo,
            "ones_r": np.ones((1,), np.float32),
            "ones_b": np.ones((1,), bf16),
        })
    return in_maps


def kernel(x_q, qkv_w, qkv_b, out_w, out_b):
    import os
    os.environ["BASS_NEVER_TRACE"] = "1"  # axon NTFF hook module is absent here
    x_q = np.asarray(x_q, dtype=np.float32)
    qkv_w = np.asarray(qkv_w, dtype=np.float32)
    qkv_b = np.asarray(qkv_b, dtype=np.float32)
    out_w = np.asarray(out_w, dtype=np.float32)
    out_b = np.asarray(out_b, dtype=np.float32)

    nc = _get_nc()
    in_maps = _shard_inputs(x_q, qkv_w, qkv_b, out_w)
    res = bass_utils.run_bass_kernel_spmd(nc, in_maps, core_ids=list(range(NCORES)))
    parts = [r["y"] for r in res.results]
    y = np.empty((B, S, E), dtype=np.float32)
    for b in range(B):
        y[b] = parts[2 * b] + parts[2 * b + 1] + out_b
    return y
